# revision 1
# baseline (speedup 1.0000x reference)
"""Wilson-Dirac operator on Trainium2, 8 NeuronCores, T-axis domain decomposition.

Lattice 24x24x24x48, complex64 field [X,Y,Z,T,3,4], gauge [4,X,Y,Z,T,3,3].
Sharding: T split into 8 slabs of 6, 1-site halos built host-side (periodic).
Host pre-scales the gauge field by -0.5 (the hopping prefactor) and ships it
twice in direction-specific layouts so every VectorE operand streams with
innermost stride <= 2 elements (strides >= 12B measured 1.25-1.6x slower):

  fh    [X*Y, Z, TS+2, s4, c3, ri2]  field slab, t-halo inline
  ghb   [4, X*Y, Z, TS+1, ri2, b3, a3]  -0.5*U[a,b] at [ri][b][a]  (backward)
  ghf   [4, X*Y, Z, TS+1, ri2, a3, b3]  -0.5*U[a,b] at [ri][a][b]  (forward,
        read transposed as U[b_out,a_out] with steps (3,1))
  outp  [X*Y, Z, TS, s4, c3, ri2]

Compute: partition = (x,y) rows. Half-spinor projection h (j,b,ri), per-site
color products into P (j,th,tu,b,a), in-place b-sum, Re/Im combine into
m (j,a,ri), spin expansion into out accumulator. Shifts: x/y via shifted DMA
row loads, z via in-row AP offsets (periodic wrap split), t via inline halo.
All engine-op APs keep <= 3 free dims (walrus TENSOR3D limit) and the
one-sync-wait-per-instruction walrus limit is handled by splitting waits
onto NoOps at BIR-json level (_split_waits_json)."""

import numpy as np

# ---------------------------------------------------------------- constants
X = Y = Z = 24
T = 48
NCORES = 8
TS = T // NCORES
MASSP4 = 4.5

# h_j = psi[j] + c_j * psi[B_j]; expansion: out[0]+=m[0], out[1]+=m[1],
# out[2] += d0*m[e0], out[3] += d1*m[e1].  Backward: c -> -c, d -> -d.
DIRSPEC = {
    0: dict(B=(3, 2), c=(-1j, -1j), e=(1, 0), d=(+1j, +1j)),
    1: dict(B=(3, 2), c=(-1, +1),   e=(1, 0), d=(+1, -1)),
    2: dict(B=(2, 3), c=(-1j, +1j), e=(0, 1), d=(+1j, -1j)),
    3: dict(B=(2, 3), c=(+1, +1),   e=(0, 1), d=(+1, +1)),
}

_CACHE = {}


def _z_splits(z0, z1, dz, Zn):
    """out z-range [z0,z1) reading input at z+dz (periodic). -> [(oz, n, iz)]"""
    if dz == 0:
        return [(z0, z1 - z0, z0)]
    if dz == -1:
        if z0 == 0:
            out = [(0, 1, Zn - 1)]
            if z1 > 1:
                out.append((1, z1 - 1, 0))
            return out
        return [(z0, z1 - z0, z0 - 1)]
    if dz == +1:
        if z1 == Zn:
            out = []
            if Zn - 1 > z0:
                out.append((z0, Zn - 1 - z0, z0 + 1))
            out.append((Zn - 1, 1, 0))
            return out
        return [(z0, z1 - z0, z0 + 1)]
    raise ValueError(dz)


def _split_waits_json(raw: bytes) -> bytes:
    """Walrus here allows only ONE sync-wait per instruction. Keep the last
    wait on the instruction, hoist the rest onto NoOps inserted immediately
    before it (same engine, semaphores monotonic => exact)."""
    import json
    bj = json.loads(raw)
    nid = 0
    for fn in bj.get("functions", []):
        for bb in fn.get("blocks", []):
            out = []
            changed = False
            for inst in bb.get("instructions", []):
                si = inst.get("sync_info")
                ow = (si or {}).get("on_wait") or []
                if len(ow) > 1:
                    changed = True
                    for w in ow[:-1]:
                        nid += 1
                        out.append({
                            "engine": inst["engine"], "ins": [], "outs": [],
                            "name": f"WSPL-{nid}", "opcode": "NoOp",
                            "sync_info": {"on_update": [], "on_wait": [w]},
                        })
                    si["on_wait"] = [ow[-1]]
                out.append(inst)
            if changed:
                bb["instructions"] = out
    return json.dumps(bj).encode()


def _install_json_wait_fix():
    import concourse.bass as bass
    if getattr(bass.Bass, "_wd_wait_fix", False):
        return
    orig = bass.Bass.to_json_bytes

    def patched(self, *a, **k):
        return _split_waits_json(orig(self, *a, **k))

    bass.Bass.to_json_bytes = patched
    bass.Bass._wd_wait_fix = True


def build_module(Xl, Yl, Zl, TSl, n_zsplit=2, nxc_override=None):
    import concourse.bass as bass
    import concourse.mybir as mybir
    from concourse.ap import AP
    from concourse.mybir import AluOpType
    from concourse.tile import TileContext

    _install_json_wait_fix()

    F32 = mybir.dt.float32
    TH = TSl + 2
    TG = TSl + 1
    XY = Xl * Yl
    NSP = 24
    NSU = 18

    nc = bass.Bass()
    fh = nc.declare_dram_parameter("fh", [XY, Zl, TH, NSP], F32, isOutput=False)
    ghb = nc.declare_dram_parameter("ghb", [4, XY, Zl, TG, NSU], F32, isOutput=False)
    ghf = nc.declare_dram_parameter("ghf", [4, XY, Zl, TG, NSU], F32, isOutput=False)
    outp = nc.declare_dram_parameter("outp", [XY, Zl, TSl, NSP], F32, isOutput=True)

    NXC = nxc_override or max(1, 128 // Yl)
    if n_zsplit > 1 and Zl % n_zsplit == 0:
        zh = Zl // n_zsplit
        zparts = [(i * zh, (i + 1) * zh) for i in range(n_zsplit)]
    else:
        zparts = [(0, Zl)]

    def sap(t, off, dims):
        return AP(t.tensor, t.offset + off, [list(t.ap[0])] + [list(d) for d in dims])

    with TileContext(nc) as tc:
        ctx_pool = tc.tile_pool(name="work", bufs=1)
        pool = ctx_pool.__enter__()
        V = nc.vector
        for x0 in range(0, Xl, NXC):
            nx = min(NXC, Xl - x0)
            R = nx * Yl
            r0 = x0 * Yl

            psi_al = pool.tile([R, Zl * TH * NSP], F32, tag="psi_al", bufs=1)
            out_t = pool.tile([R, Zl * TSl * NSP], F32, tag="out_t", bufs=1)
            # strides (field site-block = (s4, c3, ri2))
            SA = dict(z=TH * NSP, t=NSP, s=6, c=2, ri=1)   # psi_al
            SS = dict(z=TSl * NSP, t=NSP, s=6, c=2, ri=1)  # psi shifted
            SO = dict(z=TSl * NSP, t=NSP, s=6, c=2, ri=1)  # out
            SH = dict(z=TSl * 12, t=12, j=6, b=2, ri=1)    # h
            SU = dict(z=TSl * NSU, t=NSU, ri=9, r3=3, c1=1)  # gauge tiles
            SP_ = dict(z=TSl * 72, t=72, j=36, th=18, tu=9, b=3, a=1)
            SM = dict(z=TSl * 12, t=12, j=6, a=2, ri=1)    # m

            nc.gpsimd.dma_start(out=psi_al[:], in_=fh[r0:r0 + R])

            def load_x(tag, src_tensor, mu, drow, tsl0, tsl1, nreals):
                tl = pool.tile([R, Zl * (tsl1 - tsl0) * nreals], F32, tag=tag,
                               bufs=(8 if tag == "g_al" else 4 if tag == "psi_sh" else 2))
                rs = (r0 + drow) % XY
                if src_tensor is None:
                    src = lambda a, b: fh[a:b, :, tsl0:tsl1]
                else:
                    src = lambda a, b: src_tensor[mu, a:b, :, tsl0:tsl1]
                if rs + R <= XY:
                    nc.gpsimd.dma_start(out=tl[:], in_=src(rs, rs + R))
                else:
                    n1 = XY - rs
                    nc.gpsimd.dma_start(out=tl[0:n1], in_=src(rs, XY))
                    nc.gpsimd.dma_start(out=tl[n1:R], in_=src(0, R - n1))
                return tl

            def load_y(tag, src_tensor, mu, dy, tsl0, tsl1, nreals):
                tl = pool.tile([R, Zl * (tsl1 - tsl0) * nreals], F32, tag=tag,
                               bufs=(8 if tag == "g_al" else 4 if tag == "psi_sh" else 2))
                if src_tensor is None:
                    src = lambda a, b: fh[a:b, :, tsl0:tsl1]
                else:
                    src = lambda a, b: src_tensor[mu, a:b, :, tsl0:tsl1]
                for g in range(nx):
                    xa = x0 + g
                    if dy == +1:
                        nc.sync.dma_start(out=tl[g * Yl:g * Yl + Yl - 1],
                                          in_=src(xa * Yl + 1, xa * Yl + Yl))
                        nc.sync.dma_start(out=tl[g * Yl + Yl - 1:g * Yl + Yl],
                                          in_=src(xa * Yl, xa * Yl + 1))
                    else:
                        nc.sync.dma_start(out=tl[g * Yl + 1:g * Yl + Yl],
                                          in_=src(xa * Yl, xa * Yl + Yl - 1))
                        nc.sync.dma_start(out=tl[g * Yl:g * Yl + 1],
                                          in_=src(xa * Yl + Yl - 1, xa * Yl + Yl))
                return tl

            def load_g(src_tensor, mu, tsl0, tsl1):
                tl = pool.tile([R, Zl * TSl * NSU], F32, tag="g_al", bufs=8)
                nc.gpsimd.dma_start(out=tl[:], in_=src_tensor[mu, r0:r0 + R, :, tsl0:tsl1])
                return tl

            # mass term (ACT)
            nc.scalar.mul(
                sap(out_t, 0, [[SO["z"], Zl], [NSP, TSl], [1, NSP]]),
                sap(psi_al, NSP, [[SA["z"], Zl], [NSP, TSl], [1, NSP]]),
                MASSP4)

            for mu in (2, 3, 0, 1):
                spec = DIRSPEC[mu]
                # gauge tiles: fwd from ghf (transposed-read layout), bwd from ghb
                if mu == 0:
                    g_fwd = load_x("g_al", ghf, 0, -Yl, 1, TSl + 1, NSU)
                    g_bwd = load_g(ghb, 0, 1, TSl + 1)
                    psi_f = load_x("psi_sh", None, None, -Yl, 1, TSl + 1, NSP)
                    psi_b = load_x("psi_sh", None, None, +Yl, 1, TSl + 1, NSP)
                elif mu == 1:
                    g_fwd = load_y("g_al", ghf, 1, -1, 1, TSl + 1, NSU)
                    g_bwd = load_g(ghb, 1, 1, TSl + 1)
                    psi_f = load_y("psi_sh", None, None, -1, 1, TSl + 1, NSP)
                    psi_b = load_y("psi_sh", None, None, +1, 1, TSl + 1, NSP)
                elif mu == 2:
                    g_fwd = load_g(ghf, 2, 1, TSl + 1)
                    g_bwd = load_g(ghb, 2, 1, TSl + 1)
                else:
                    g_fwd = load_g(ghf, 3, 0, TSl)
                    g_bwd = load_g(ghb, 3, 1, TSl + 1)

                for sgn in (+1, -1):
                    fwd = sgn == +1
                    cj = spec["c"] if fwd else tuple(-v for v in spec["c"])
                    dj = spec["d"] if fwd else tuple(-v for v in spec["d"])

                    if mu <= 1:
                        psit, dzp, toffp, SPS = (psi_f if fwd else psi_b), 0, 0, SS
                    elif mu == 2:
                        psit, dzp, toffp, SPS = psi_al, (-1 if fwd else +1), NSP, SA
                    else:
                        psit, dzp, toffp, SPS = psi_al, 0, (0 if fwd else 2 * NSP), SA

                    # --- projection into h (j, b, ri); psi innermost (c,ri)
                    ht = pool.tile([R, Zl * TSl * 12], F32, tag="h", bufs=1)
                    for j in (0, 1):
                        A, B, c = j, spec["B"][j], cj[j]
                        for (oz, nz, iz) in _z_splits(0, Zl, dzp, Zl):
                            hbase = oz * SH["z"] + j * SH["j"]
                            pb = iz * SPS["z"] + toffp
                            zt = [[SPS["z"], nz], [SPS["t"], TSl]]
                            hzt = [[SH["z"], nz], [SH["t"], TSl]]
                            if c.imag == 0.0:
                                op = AluOpType.add if c.real > 0 else AluOpType.subtract
                                V.tensor_tensor(
                                    sap(ht, hbase, hzt + [[1, 6]]),
                                    sap(psit, pb + A * 6, zt + [[1, 6]]),
                                    sap(psit, pb + B * 6, zt + [[1, 6]]),
                                    op)
                            else:
                                sg = 1.0 if c.imag > 0 else -1.0
                                # h_re = psiA_re - sg*psiB_im ; h_im = psiA_im + sg*psiB_re
                                V.tensor_tensor(
                                    sap(ht, hbase, hzt + [[SH["b"], 3]]),
                                    sap(psit, pb + A * 6, zt + [[SPS["c"], 3]]),
                                    sap(psit, pb + B * 6 + 1, zt + [[SPS["c"], 3]]),
                                    AluOpType.subtract if sg > 0 else AluOpType.add)
                                V.tensor_tensor(
                                    sap(ht, hbase + 1, hzt + [[SH["b"], 3]]),
                                    sap(psit, pb + A * 6 + 1, zt + [[SPS["c"], 3]]),
                                    sap(psit, pb + B * 6, zt + [[SPS["c"], 3]]),
                                    AluOpType.add if sg > 0 else AluOpType.subtract)

                    gt = g_fwd if fwd else g_bwd
                    dzu = -1 if (fwd and mu == 2) else 0

                    for (zl0, zl1) in zparts:
                        hz = zl1 - zl0
                        pt = pool.tile([R, hz * TSl * 72], F32, tag="P", bufs=1)
                        mt = pool.tile([R, hz * TSl * 12], F32, tag="m", bufs=1)

                        # --- products: per (j,th,tu): P[zt,(b,a)] = U' * h
                        # out/in0 innermost stride 1, in1 broadcast over a
                        for j in (0, 1):
                            for th in (0, 1):
                                for tu in (0, 1):
                                    for (oz, nz, iz) in _z_splits(zl0, zl1, dzu, Zl):
                                        po = (oz - zl0) * SP_["z"] + j * SP_["j"] + th * SP_["th"] + tu * SP_["tu"]
                                        V.tensor_tensor(
                                            sap(pt, po, [[SP_["t"], nz * TSl], [SP_["b"], 3], [1, 3]]),
                                            sap(gt, iz * SU["z"] + tu * SU["ri"], [[SU["t"], nz * TSl], [3, 3], [1, 3]]),
                                            sap(ht, oz * SH["z"] + j * SH["j"] + th, [[SH["t"], nz * TSl], [SH["b"], 3], [0, 3]]),
                                            AluOpType.mult)

                        # --- b-sum in place: P[b0] += P[b1]; P[b0] += P[b2]
                        bdims = [[SP_["t"], hz * TSl], [SP_["tu"], 8], [1, 3]]
                        V.tensor_tensor(sap(pt, 0, bdims), sap(pt, 0, bdims),
                                        sap(pt, SP_["b"], bdims), AluOpType.add)
                        V.tensor_tensor(sap(pt, 0, bdims), sap(pt, 0, bdims),
                                        sap(pt, 2 * SP_["b"], bdims), AluOpType.add)
                        # --- combine into m (j, a, ri):
                        # m_re = P[rr] +- P[ii]; m_im = P[ir] -+ P[ri]
                        cdims = [[SP_["t"], hz * TSl], [SP_["j"], 2], [1, 3]]
                        mdims = [[SM["t"], hz * TSl], [SM["j"], 2], [SM["a"], 3]]
                        RR, II = 0, SP_["th"] + SP_["tu"]
                        IR, RI = SP_["th"], SP_["tu"]
                        V.tensor_tensor(sap(mt, 0, mdims), sap(pt, RR, cdims), sap(pt, II, cdims),
                                        AluOpType.add if fwd else AluOpType.subtract)
                        V.tensor_tensor(sap(mt, 1, mdims), sap(pt, IR, cdims), sap(pt, RI, cdims),
                                        AluOpType.subtract if fwd else AluOpType.add)

                        # --- expansion into out_t (s,c,ri layout; (c,ri)=[1,6])
                        ob = zl0 * SO["z"]
                        ozt = [[NSP, hz * TSl]]
                        mzt = [[SM["t"], hz * TSl]]
                        for s in (0, 1):
                            os_ = sap(out_t, ob + s * SO["s"], ozt + [[1, 6]])
                            V.tensor_tensor(os_, os_, sap(mt, s * SM["j"], mzt + [[1, 6]]),
                                            AluOpType.add)
                        for si_, (ei, dv) in enumerate(zip(spec["e"], dj)):
                            sb = ob + (2 + si_) * SO["s"]
                            if dv.imag == 0.0:
                                op = AluOpType.add if dv.real > 0 else AluOpType.subtract
                                os_ = sap(out_t, sb, ozt + [[1, 6]])
                                V.tensor_tensor(os_, os_, sap(mt, ei * SM["j"], mzt + [[1, 6]]), op)
                            else:
                                sg = 1.0 if dv.imag > 0 else -1.0
                                # out_re += -sg*m_im ; out_im += sg*m_re
                                ore = sap(out_t, sb, ozt + [[SO["c"], 3]])
                                V.tensor_tensor(ore, ore,
                                                sap(mt, ei * SM["j"] + 1, mzt + [[SM["a"], 3]]),
                                                AluOpType.subtract if sg > 0 else AluOpType.add)
                                oim = sap(out_t, sb + 1, ozt + [[SO["c"], 3]])
                                V.tensor_tensor(oim, oim,
                                                sap(mt, ei * SM["j"], mzt + [[SM["a"], 3]]),
                                                AluOpType.add if sg > 0 else AluOpType.subtract)

            nc.gpsimd.dma_start(out=outp[r0:r0 + R], in_=out_t[:])
        ctx_pool.__exit__(None, None, None)
    return nc


# ---------------------------------------------------------------- host side
def _prep_core_inputs(fv, gv, t0, Xl, Yl, Zl, Tl, TSl):
    """fv: [X,Y,Z,T,3,4,2] f32 view (c,s,ri). gv: [4,X,Y,Z,T,3,3,2] (a,b,ri).
    Returns fh [XY,Z,TH,(s,c,ri)], ghb [...,(ri,b,a)], ghf [...,(ri,a,b)],
    gauge pre-scaled by -0.5."""
    idx = [(t0 - 1) % Tl] + [(t0 + i) % Tl for i in range(TSl)] + [(t0 + TSl) % Tl]
    f = fv[:, :, :, idx]                       # [X,Y,Z,TH,c,s,ri]
    f = f.transpose(0, 1, 2, 3, 5, 4, 6)       # -> (s,c,ri)
    fhn = np.ascontiguousarray(f).reshape(Xl * Yl, Zl, TSl + 2, 24)
    idg = [(t0 - 1 + i) % Tl for i in range(TSl + 1)]
    g = gv[:, :, :, :, idg]                    # [4,X,Y,Z,TG,a,b,ri]
    ghfn = np.ascontiguousarray(g.transpose(0, 1, 2, 3, 4, 7, 5, 6))  # (ri,a,b)
    ghbn = np.ascontiguousarray(g.transpose(0, 1, 2, 3, 4, 7, 6, 5))  # (ri,b,a)
    ghfn *= -0.5
    ghbn *= -0.5
    return (fhn, ghfn.reshape(4, Xl * Yl, Zl, TSl + 1, 18),
            ghbn.reshape(4, Xl * Yl, Zl, TSl + 1, 18))


def _out_to_complex(o, Xl, Yl, Zl, TSl):
    o = o.reshape(Xl, Yl, Zl, TSl, 4, 3, 2)    # (s,c,ri)
    o = o.transpose(0, 1, 2, 3, 5, 4, 6)       # -> (c,s,ri)
    return o[..., 0] + 1j * o[..., 1]


def kernel(field, gauge_field):
    from concourse.bass_utils import run_bass_kernel_spmd

    key = "full"
    if key not in _CACHE:
        _CACHE[key] = build_module(X, Y, Z, TS)
    nc = _CACHE[key]

    fv = np.ascontiguousarray(field).view(np.float32).reshape(X, Y, Z, T, 3, 4, 2)
    gv = np.ascontiguousarray(gauge_field).view(np.float32).reshape(4, X, Y, Z, T, 3, 3, 2)

    in_maps = []
    for k in range(NCORES):
        fhn, ghfn, ghbn = _prep_core_inputs(fv, gv, k * TS, X, Y, Z, T, TS)
        in_maps.append({"fh": fhn, "ghf": ghfn, "ghb": ghbn})

    res = run_bass_kernel_spmd(nc, in_maps, list(range(NCORES))).results

    out = np.empty((X, Y, Z, T, 3, 4), np.complex64)
    for k in range(NCORES):
        out[:, :, :, k * TS:(k + 1) * TS] = _out_to_complex(
            res[k]["outp"], X, Y, Z, TS)
    return out



# revision 8
# speedup vs baseline: 2.9693x; 2.9693x over previous
"""Wilson-Dirac on 8 trn2 cores — comp-major bf16 layout, multi-engine.

T-axis domain decomposition (TS=6/core, halos host-built). Host ships all
tensors in component-major bf16 planes: per (x,y)-row partition, each of the
24 psi / 27 gauge components is a contiguous (t, z) plane, so every engine op
streams with innermost step-1 over >=144 elements -> DVE 2x_1P bf16 mode.

Per direction d (8 = 4 mu x fwd/bwd), with U_d = -0.5 * (U_mu shifted,
daggered as needed) prebaked host-side (planes (tu,b,a) holding U_d[a][b],
tu in {re, im, -im}):
  DVE:     h = proj(psi_shifted)        (12 planes)
           P[q,j,b,a] = U_plane * h     (72 planes, stride-0 mid broadcast)
           bsum: P[..,b0,..] += b1,b2   (a-planes now hold m contributions)
  TensorE: out_psum[(ri,s) bank] += +-I @ P-planes  (16 matmuls/dir, fp32
           PSUM accumulation; mass term = 4.5*I @ psi)
  ScalarE: evict PSUM -> SBUF bf16; DMA out.
x/y shifts come from host-pre-rolled DRAM copies (contiguous loads on the
gpsimd/sync queues); z/t shifts are in-plane AP offsets (z wrap split)."""

import os

import numpy as np

X = Y = Z = 24
T = 48
NCORES = 8
TS = T // NCORES
XY = X * Y
PS = 144          # plane size for central tiles: t6 * z24 (z inner)
PSA = 192         # pa plane: t8 * z24
NXC = 5

# h_j = psi[A_j] + c_j * psi[B_j]; A=(0,1). out[2+si] += d_si * m[e_si].
# bwd: c -> -c, d -> -d.
DIRSPEC = {
    0: dict(B=(3, 2), c=(-1j, -1j), e=(1, 0), d=(+1j, +1j)),
    1: dict(B=(3, 2), c=(-1, +1),   e=(1, 0), d=(+1, -1)),
    2: dict(B=(2, 3), c=(-1j, +1j), e=(0, 1), d=(+1j, -1j)),
    3: dict(B=(2, 3), c=(+1, +1),   e=(0, 1), d=(+1, +1)),
}
DIRS = [(3, +1), (3, -1), (2, +1), (2, -1), (0, +1), (0, -1), (1, +1), (1, -1)]

_CACHE = {}


def _split_waits_json(raw: bytes) -> bytes:
    """Walrus allows only ONE sync-wait per instruction: hoist extras onto
    NoOps inserted immediately before (same engine; exact). Also drop
    redundant consecutive PE Ldweights with identical source (the PE array
    already holds those weights); their sync_info moves onto a NoOp."""
    import json
    import os
    bj = json.loads(raw)
    nid = 0
    dedup = not os.environ.get("WD_NO_LDW_DEDUP")
    for fn in bj.get("functions", []):
        for bb in fn.get("blocks", []):
            out = []
            changed = False
            last_ldw = None
            for inst in bb.get("instructions", []):
                if dedup and inst["engine"] == "PE":
                    if inst["opcode"] == "Ldweights":
                        key = json.dumps(inst["ins"], sort_keys=True)
                        if key == last_ldw:
                            si0 = inst.get("sync_info") or {}
                            if (si0.get("on_wait") or si0.get("on_update")):
                                inst = {"engine": "PE", "ins": [], "outs": [],
                                        "name": inst["name"], "opcode": "NoOp",
                                        "sync_info": si0}
                                changed = True
                            else:
                                changed = True
                                continue
                        else:
                            last_ldw = key
                    elif inst["opcode"] not in ("Matmult", "NoOp"):
                        last_ldw = None
                si = inst.get("sync_info")
                ow = (si or {}).get("on_wait") or []
                if len(ow) > 1:
                    changed = True
                    for w in ow[:-1]:
                        nid += 1
                        out.append({
                            "engine": inst["engine"], "ins": [], "outs": [],
                            "name": f"WSPL-{nid}", "opcode": "NoOp",
                            "sync_info": {"on_update": [], "on_wait": [w]},
                        })
                    si["on_wait"] = [ow[-1]]
                out.append(inst)
            if changed:
                bb["instructions"] = out
    return json.dumps(bj).encode()


def _install_json_wait_fix():
    import concourse.bass as bass
    if getattr(bass.Bass, "_wd_wait_fix", False):
        return
    orig = bass.Bass.to_json_bytes

    def patched(self, *a, **k):
        return _split_waits_json(orig(self, *a, **k))

    bass.Bass.to_json_bytes = patched
    bass.Bass._wd_wait_fix = True


def build_module():
    import concourse.bass as bass
    import concourse.mybir as mybir
    from concourse.ap import AP
    from concourse.mybir import AluOpType
    from concourse.tile import TileContext

    _install_json_wait_fix()
    BF = mybir.dt.bfloat16
    F32 = mybir.dt.float32

    nc = bass.Bass()
    pa_d = nc.declare_dram_parameter("pa", [XY, 24 * PSA], BF, isOutput=False)
    psh_d = {}
    for nm in ("pxf", "pxb", "pyf", "pyb"):
        psh_d[nm] = nc.declare_dram_parameter(nm, [XY, 24 * PS], BF, isOutput=False)
    g_d = [nc.declare_dram_parameter(f"g{i}", [XY, 27 * PS], BF, isOutput=False)
           for i in range(8)]
    id_d = nc.declare_dram_parameter("iden", [128, 3 * 128], BF, isOutput=False)
    out_d = nc.declare_dram_parameter("outp", [XY, 24 * PS], BF, isOutput=True)

    def sap(t, off, dims):
        return AP(t.tensor, t.offset + off, [list(t.ap[0])] + [list(d) for d in dims])

    with TileContext(nc) as tc:
        ctx_pool = tc.tile_pool(name="work", bufs=1)
        pool = ctx_pool.__enter__()
        ctx_ps = tc.tile_pool(name="ps", bufs=1, space="PSUM")
        ppool = ctx_ps.__enter__()
        V = nc.vector

        idt = pool.tile([128, 3 * 128], BF, tag="iden", bufs=1)
        nc.gpsimd.dma_start(out=idt[:], in_=id_d[:])

        for ci, x0 in enumerate(range(0, X, NXC)):
            nx = min(NXC, X - x0)
            R = nx * Y
            r0 = x0 * Y
            pI = idt[0:R, 0:R]
            nI = idt[0:R, 128:128 + R]
            hI = idt[0:R, 256:256 + R]

            pa = pool.tile([R, 24 * PSA], BF, tag="pa", bufs=2)
            nc.gpsimd.dma_start(out=pa[:], in_=pa_d[r0:r0 + R])
            psh = {}
            for qi, nm in enumerate(("pxf", "pxb", "pyf", "pyb")):
                psh[nm] = pool.tile([R, 24 * PS], BF, tag=nm, bufs=1, name=nm)
                eng = nc.sync if qi % 2 else nc.gpsimd
                eng.dma_start(out=psh[nm][:], in_=psh_d[nm][r0:r0 + R])
            gt = []
            for i in range(8):
                g = pool.tile([R, 27 * PS], BF, tag="g", bufs=9, name="g")
                eng = nc.sync if i % 2 else nc.gpsimd
                eng.dma_start(out=g[:], in_=g_d[i][r0:r0 + R])
                gt.append(g)
            psum = ppool.tile([R, 4096], F32, tag="ps", bufs=1)
            outsb = pool.tile([R, 24 * PS], BF, tag="osb", bufs=2)

            NDIRMM = int(os.environ.get("WD_NDIRMM", "8"))
            NO_DIRMM = NDIRMM == 0
            # ---- mass: psum[(ri,s) bank] = 4.5 * psi  (start accumulation)
            for g8 in range(8):  # g8 = ri*4+s
                nc.tensor.matmul(
                    sap(psum, g8 * 512, [[1, 432]]), hI,
                    sap(pa, (g8 * 3) * PSA + 24, [[PSA, 3], [1, PS]]),
                    start=True, stop=NO_DIRMM)

            for di, (mu, sgn) in enumerate(DIRS):
                spec = DIRSPEC[mu]
                fwd = sgn > 0
                cj = spec["c"] if fwd else tuple(-v for v in spec["c"])
                dj = spec["d"] if fwd else tuple(-v for v in spec["d"])
                B = spec["B"]

                # psi source tile + site offsets/parts
                if mu == 3:
                    psit, psz = pa, PSA
                    parts = [(0, 0 if fwd else 48, [[1, PS]], [[1, PS]])]
                elif mu == 2:
                    psit, psz = pa, PSA
                    if fwd:   # out z <- z-1
                        parts = [(1, 24 + 0, [[24, 6], [1, 23]], [[24, 6], [1, 23]]),
                                 (0, 24 + 23, [[24, 6], [1, 1]], [[24, 6], [1, 1]])]
                    else:     # out z <- z+1
                        parts = [(0, 24 + 1, [[24, 6], [1, 23]], [[24, 6], [1, 23]]),
                                 (23, 24 + 0, [[24, 6], [1, 1]], [[24, 6], [1, 1]])]
                else:
                    psit, psz = psh[{(0, True): "pxf", (0, False): "pxb",
                                     (1, True): "pyf", (1, False): "pyb"}[(mu, fwd)]], PS
                    parts = [(0, 0, [[1, PS]], [[1, PS]])]

                # ---- projection -> h (planes th*6+j*3+b)
                ht = pool.tile([R, 12 * PS], BF, tag="h", bufs=2)
                for j in (0, 1):
                    c = cj[j]
                    for (hoff, poff, hdims, pdims) in parts:
                        hb = lambda th: (th * 6 + j * 3) * PS + hoff
                        pp = lambda ri, s: (ri * 12 + s * 3) * psz + poff
                        pdim = [[psz, 3]] + pdims
                        hdim = [[PS, 3]] + hdims
                        if c.imag == 0.0:
                            op = AluOpType.add if c.real > 0 else AluOpType.subtract
                            for th in (0, 1):
                                V.tensor_tensor(
                                    sap(ht, hb(th), hdim),
                                    sap(psit, pp(th, j), pdim),
                                    sap(psit, pp(th, B[j]), pdim), op)
                        else:
                            sg = c.imag > 0
                            # h_re = psiA_re -s psiB_im ; h_im = psiA_im +s psiB_re
                            V.tensor_tensor(
                                sap(ht, hb(0), hdim),
                                sap(psit, pp(0, j), pdim),
                                sap(psit, pp(1, B[j]), pdim),
                                AluOpType.subtract if sg else AluOpType.add)
                            V.tensor_tensor(
                                sap(ht, hb(1), hdim),
                                sap(psit, pp(1, j), pdim),
                                sap(psit, pp(0, B[j]), pdim),
                                AluOpType.add if sg else AluOpType.subtract)

                # ---- products: P[q,j,b,a] planes; q:(th,tu_eff):
                # q0:(0,0) q1:(1,0) q2:(0,1) q3:(1,2=-u_im)
                pt = pool.tile([R, 72 * PS], BF, tag="P", bufs=2)
                gtile = gt[di]
                for q, (th, tu) in enumerate(((0, 0), (1, 0), (0, 1), (1, 2))):
                    for j in (0, 1):
                        V.tensor_tensor(
                            sap(pt, (q * 18 + j * 9) * PS,
                                [[3 * PS, 3], [PS, 3], [1, PS]]),
                            sap(gtile, (tu * 9) * PS,
                                [[3 * PS, 3], [PS, 3], [1, PS]]),
                            sap(ht, (th * 6 + j * 3) * PS,
                                [[PS, 3], [0, 3], [1, PS]]),
                            AluOpType.mult)

                # ---- bsum: P[qj, b0, a] += P[qj, b1, a]; += P[qj, b2, a]
                bd = [[9 * PS, 8], [PS, 3], [1, PS]]
                V.tensor_tensor(sap(pt, 0, bd), sap(pt, 0, bd),
                                sap(pt, 3 * PS, bd), AluOpType.add)
                V.tensor_tensor(sap(pt, 0, bd), sap(pt, 0, bd),
                                sap(pt, 6 * PS, bd), AluOpType.add)

                # ---- TensorE accumulation: m_re[j] = q0+q3 ; m_im[j] = q1+q2
                RE, IM = (0, 3), (1, 2)
                acc = []  # (group g8, qset j, sign)
                for s in (0, 1):
                    acc.append((0 * 4 + s, RE, s, +1))
                    acc.append((1 * 4 + s, IM, s, +1))
                for si in (0, 1):
                    jj, dv = spec["e"][si], dj[si]
                    s = 2 + si
                    if dv.imag == 0.0:
                        sg = +1 if dv.real > 0 else -1
                        acc.append((0 * 4 + s, RE, jj, sg))
                        acc.append((1 * 4 + s, IM, jj, sg))
                    else:
                        sg = +1 if dv.imag > 0 else -1
                        acc.append((0 * 4 + s, IM, jj, -sg))
                        acc.append((1 * 4 + s, RE, jj, sg))
                last = di == min(NDIRMM, len(DIRS)) - 1
                # emit +1 stationary first, then -1 (fewer eventual reloads)
                for want in (+1, -1):
                    for (g8, qs, j, sign) in acc:
                        if sign != want:
                            continue
                        for k, q in enumerate(qs):
                            if di >= NDIRMM:
                                continue
                            nc.tensor.matmul(
                                sap(psum, g8 * 512, [[1, 432]]),
                                pI if sign > 0 else nI,
                                sap(pt, (q * 18 + j * 9) * PS, [[PS, 3], [1, PS]]),
                                start=False, stop=(last and k == 1))

            # ---- evict + store
            nc.scalar.copy(sap(outsb, 0, [[432, 8], [1, 432]]),
                           sap(psum, 0, [[512, 8], [1, 432]]))
            nc.gpsimd.dma_start(out=out_d[r0:r0 + R], in_=outsb[:])

        ctx_ps.__exit__(None, None, None)
        ctx_pool.__exit__(None, None, None)
    return nc


# ------------------------------------------------------------------ host side
def _prep_core_inputs(fv, gv, t0):
    """fv [X,Y,Z,T,c,s,ri] f32; gv [4,X,Y,Z,T,a,b,ri] f32 -> per-core dict."""
    import ml_dtypes
    bf = ml_dtypes.bfloat16
    t8 = [(t0 - 1 + i) % T for i in range(TS + 2)]
    t6 = [t0 + i for i in range(TS)]

    def pack_psi(f):  # [X,Y,Z,t,c,s,ri] -> [XY, (ri,s,c), t, z]
        nt = f.shape[3]
        return np.ascontiguousarray(f.transpose(0, 1, 6, 5, 4, 3, 2)).reshape(
            XY, 24 * nt * Z).astype(bf)

    out = {"pa": pack_psi(fv[:, :, :, t8])}
    base = fv[:, :, :, t6]
    out["pxf"] = pack_psi(np.roll(base, +1, axis=0))
    out["pxb"] = pack_psi(np.roll(base, -1, axis=0))
    out["pyf"] = pack_psi(np.roll(base, +1, axis=1))
    out["pyb"] = pack_psi(np.roll(base, -1, axis=1))

    for di, (mu, sgn) in enumerate(DIRS):
        if sgn > 0:
            arr = np.roll(gv[mu], 1, axis=mu)[:, :, :, t6]
            # UT[tu,b,a] = U_dir[a][b] = -0.5*conj(arr[b][a])
            ut = arr.transpose(0, 1, 6, 4, 5, 3, 2).copy()  # [X,Y,tu,b,a,t,z]
            ut[:, :, 0] *= -0.5
            ut[:, :, 1] *= +0.5
        else:
            arr = gv[mu][:, :, :, t6]
            # UT[tu,b,a] = -0.5*arr[a][b]
            ut = arr.transpose(0, 1, 6, 5, 4, 3, 2).copy()
            ut *= -0.5
        g = np.empty((X, Y, 3, 3, 3, TS, Z), np.float32)
        g[:, :, :2] = ut
        g[:, :, 2] = -ut[:, :, 1]
        out[f"g{di}"] = np.ascontiguousarray(g).reshape(XY, 27 * PS).astype(bf)

    iden = np.zeros((128, 3 * 128), np.float32)
    iden[:, 0:128] = np.eye(128)
    iden[:, 128:256] = -np.eye(128)
    iden[:, 256:384] = 4.5 * np.eye(128)
    out["iden"] = iden.astype(bf)
    return out


def kernel(field, gauge_field):
    from concourse.bass_utils import run_bass_kernel_spmd

    if "nc" not in _CACHE:
        _CACHE["nc"] = build_module()
    nc = _CACHE["nc"]

    fv = np.ascontiguousarray(field).view(np.float32).reshape(X, Y, Z, T, 3, 4, 2)
    gv = np.ascontiguousarray(gauge_field).view(np.float32).reshape(4, X, Y, Z, T, 3, 3, 2)

    in_maps = [_prep_core_inputs(fv, gv, k * TS) for k in range(NCORES)]
    res = run_bass_kernel_spmd(nc, in_maps, list(range(NCORES))).results

    out = np.empty((X, Y, Z, T, 3, 4), np.complex64)
    for k in range(NCORES):
        o = res[k]["outp"].astype(np.float32).reshape(X, Y, 2, 4, 3, TS, Z)
        oc = o[:, :, 0] + 1j * o[:, :, 1]          # [x,y,s,c,t,z]
        out[:, :, :, k * TS:(k + 1) * TS] = oc.transpose(0, 1, 5, 4, 3, 2)
    return out
